# revision 1
# baseline (speedup 1.0000x reference)
"""GATWithSentenceEmbedding Trainium2 kernel (8 NeuronCores, SPMD + collectives).

Sharding:
  - fcl [E,E] / fce [BERT,E] column-sharded (each core computes a 1024-chunk of
    g1/g2); fc2 [2E,E] row-sharded with matching rows; one AllReduce yields the
    full orig_edge_logits on every core.
  - GAT: edges sorted by dst; core c owns dst nodes [256c, 256c+256) and their
    incoming edges (incl. self-loops). Segment softmax/aggregation via one-hot
    matmul into PSUM. xp2 / h2 chunks are AllGathered between layers.
  - Edge MLP: same dst-based edge partition (real edges only); masked-BN stats
    combined with two tiny AllReduces.
"""

import numpy as np
from contextlib import ExitStack

import concourse.bass as bass
import concourse.mybir as mybir
import concourse.bass_isa as bass_isa
import concourse.tile as tile
from concourse import bacc
from concourse.bass_utils import run_bass_kernel_spmd
from concourse.masks import make_identity

N, F, HC, S, H, E, BERT = 2048, 256, 256, 512, 4, 8192, 768
NCORES = 8
P = 128
NCHUNK = N // NCORES          # 256 dst nodes per core
ECH = E // NCORES             # 1024 g1/g2 columns per core
XP1W = H * HC + 2 * H         # 1032 = xp1 | al_s | al_d
XP2W = F + 2                  # 258  = xp2 | al_s | al_d
HC2 = HC // 2                 # 128
BIG = 1.0e9
NNZ_PAD = 1280                # padded count of nonzero g rows (mean 1024, +11sig)

dt = mybir.dt
AF = mybir.ActivationFunctionType
ALU = mybir.AluOpType
RG = [list(range(NCORES))]

_cache = {}
last_in_maps = None
DEBUG = False
TRACE = False
last_results = None
PHASE_MARKS = []


def _mark(nc, phase):
    insts = nc.m.functions[0].blocks[0].instructions
    PHASE_MARKS.append((phase, len(insts), insts[-1].name if insts else None))


def _build(nt_g: int, nt_m: int, debug: bool = False, stage: int = 4):
    pad_g = nt_g * P
    pad_m = nt_m * P
    nc = bacc.Bacc("TRN2", target_bir_lowering=False, debug=False)

    def inp(name, shape, dtype=dt.float32):
        return nc.dram_tensor(name, shape, dtype, kind="ExternalInput")

    # shared inputs
    xT_in = inp("xT", [F, N])
    sent_in = inp("sent_emb", [BERT])
    elph_in = inp("elp_h", [E], dt.float16)
    elpl_in = inp("elp_l", [E], dt.float16)
    fc0_w = inp("fc0_w", [BERT, S]); fc0_b = inp("fc0_b", [S])
    fc1_w = inp("fc1_w", [F, S]); fc1_b = inp("fc1_b", [S])
    c1w = inp("conv1_W", [S, H * HC]); c1a = inp("conv1_a", [2 * H * HC])
    c1b = inp("conv1_b", [H * HC])
    c2w = inp("conv2_W", [H * HC, F]); c2a = inp("conv2_a", [2 * F])
    c2b = inp("conv2_b", [F])
    m1w = inp("mlp1_w", [4 * F, HC]); m1b = inp("mlp1_b", [HC])
    bn1g = inp("bn1_g", [HC]); bn1b = inp("bn1_b", [HC])
    m2w = inp("mlp2_w", [HC, HC2]); m2b = inp("mlp2_b", [HC2])
    bn2g = inp("bn2_g", [HC2]); bn2b = inp("bn2_b", [HC2])
    m3w = inp("mlp3_w", [HC2, 1]); m3b = inp("mlp3_b", [1])
    fc2_b = inp("fc2_b", [E])
    # per-core inputs
    fclwh_sh = inp("fclwh_sh", [E, ECH], dt.float16)
    fclwl_sh = inp("fclwl_sh", [E, ECH], dt.float16)
    fclb_sh = inp("fclb_sh", [ECH])
    fcewh_sh = inp("fcewh_sh", [BERT, ECH], dt.float16)
    fcewl_sh = inp("fcewl_sh", [BERT, ECH], dt.float16)
    fceb_sh = inp("fceb_sh", [ECH])
    fc2w_sh = inp("fc2w_sh", [2 * ECH + 1, E])   # +1 zero sentinel row
    iota1_in = inp("iota1", [2 * ECH])           # value k+1 at slot k
    posl_in = inp("pos_lin", [NNZ_PAD])          # linear slot index as f32
    g_src = inp("g_src", [pad_g], dt.int32)
    g_dst = inp("g_dst", [pad_g], dt.int32)
    g_lidx = inp("g_lidx", [pad_g], dt.int32)
    g_oh = inp("g_oh", [pad_g, NCHUNK], dt.float16)
    m_src = inp("m_src", [pad_m], dt.int32)
    m_dst = inp("m_dst", [pad_m], dt.int32)
    m_lidx = inp("m_lidx", [pad_m], dt.int32)
    # outputs
    orig_out = nc.dram_tensor("orig_out", [E], dt.float32, kind="ExternalOutput")
    kdbg_out = nc.dram_tensor("kdbg", [NNZ_PAD], dt.int32, kind="ExternalOutput")
    gdbg_out = nc.dram_tensor("gdbg", [NNZ_PAD], dt.float32, kind="ExternalOutput")
    pdbg_out = nc.dram_tensor("pdbg", [P, 2 * ECH // P], dt.float32,
                              kind="ExternalOutput")
    g12dbg_out = nc.dram_tensor("g12dbg", [2 * ECH], dt.float32,
                                kind="ExternalOutput")
    score_out = nc.dram_tensor("score_out", [pad_m], dt.float32,
                               kind="ExternalOutput")
    dbg = {}
    if debug:
        fp16_dbg = {"xp1_dbg", "xs_dbg", "ad_dbg", "h1_dbg", "xp2_dbg", "h2_dbg"}
        for nm, shp in [("xp1_dbg", [N, XP1W]),
                        ("h1_dbg", [NCHUNK, H * HC]), ("xp2_dbg", [N, XP2W]),
                        ("h2_dbg", [N, F]), ("z1_dbg", [pad_m, HC]),
                        ("st1_dbg", [520]),
                        ("xs_dbg", [pad_g, XP1W]), ("ad_dbg", [pad_g, 2 * H]),
                        ("ex_dbg", [pad_g, H]), ("den_dbg", [2 * P, H]),
                        ("msum_dbg", [2 * P, H * HC])]:
            dbg[nm] = nc.dram_tensor(
                nm, shp, dt.float16 if nm in fp16_dbg else dt.float32,
                kind="ExternalOutput")

    def bcast(dram_handle, cols, offset=0):
        """AP reading a [1, cols] DRAM row replicated over 128 partitions."""
        return bass.AP(tensor=dram_handle.ap().tensor, offset=offset,
                       ap=[[0, P], [1, cols]])

    def bcast_ap(ap_tile, cols, offset=0):
        a = ap_tile[:] if not isinstance(ap_tile, bass.AP) else ap_tile
        return bass.AP(tensor=a.tensor, offset=a.offset + offset,
                       ap=[[0, P], [1, cols]])

    with tile.TileContext(nc) as tc:
        with (
            tc.tile_pool(name="dram", bufs=1, space="DRAM") as dram,
            tc.tile_pool(name="single", bufs=1) as single,
            tc.tile_pool(name="sb", bufs=2) as sb,
            tc.tile_pool(name="psum2", bufs=2, space="PSUM") as psum2,
            tc.tile_pool(name="keep", bufs=1) as keep,
        ):
            ident = single.tile([P, P], dt.float32)
            make_identity(nc, ident[:])
            ident_h = single.tile([P, P], dt.float16)
            nc.vector.tensor_copy(ident_h[:], ident[:])

            # internal DRAM
            xp1_dram = dram.tile([N, XP1W], dt.float16)
            al1d_dram = dram.tile([N, 2 * H], dt.float16)
            fc2part = dram.tile([E], dt.float32)
            klist_dram = dram.tile([NNZ_PAD, 1], dt.float32)
            encd = dram.tile([2 * ECH, 1], dt.float32)
            graw = dram.tile([2 * ECH], dt.float32)
            msg_dram = dram.tile([nt_g * P, H * HC + H], dt.float16)
            g12ext = dram.tile([2 * ECH + 1, 1], dt.float32)
            logits_dram = dram.tile([E], dt.float32, addr_space="Shared")
            lext_dram = dram.tile([E + 2, 1], dt.float32)
            sent_dram = dram.tile([S], dt.float32)
            g12_dram = dram.tile([2 * ECH], dt.float32)
            xp2_in = dram.tile([NCHUNK, XP2W], dt.float16)
            xp2_dram = dram.tile([N, XP2W], dt.float16, addr_space="Shared")
            h2_in = dram.tile([NCHUNK, F], dt.float16)
            h2_dram = dram.tile([N, F], dt.float16, addr_space="Shared")
            st1_in = dram.tile([520], dt.float32)
            st1_ag = dram.tile([NCORES * 520], dt.float32, addr_space="Shared")
            st1_out = dram.tile([520], dt.float32)
            st2_in = dram.tile([2 * HC2], dt.float32)
            st2_ag = dram.tile([NCORES * 2 * HC2], dt.float32,
                               addr_space="Shared")
            st2_out = dram.tile([2 * HC2], dt.float32)
            row_dram = dram.tile([4 * HC], dt.float32)  # scratch rows for bcast

            # ======== phases A (h/xp1) + B (g1/g2/fc2) — scoped pools ========
            esA = ExitStack()
            sbA = esA.enter_context(tc.tile_pool(name="sbA", bufs=2))
            psA = esA.enter_context(tc.tile_pool(name="psA", bufs=1, space="PSUM"))

            # sent = relu(sent_emb @ fc0_w + fc0_b), weights-stationary chunks
            sent_sb = single.tile([P, BERT // P], dt.float32)
            nc.sync.dma_start(sent_sb[:], sent_in.ap().rearrange("(k p) -> p k", p=P))
            fc0w_t = [sbA.tile([P, S], dt.float32, tag=f"fc0w{k}", bufs=1,
                               name=f"fc0w{k}")
                      for k in range(BERT // P)]
            for k in range(BERT // P):
                nc.sync.dma_start(fc0w_t[k][:], fc0_w[k * P:(k + 1) * P, :])
            for j in range(S // P):
                ps_v = psA.tile([P, E // P], dt.float32, space="PSUM",
                                tag="vec", bufs=1, name="ps_v")
                for k in range(BERT // P):
                    nc.tensor.matmul(ps_v[:, 0:1],
                                     lhsT=fc0w_t[k][:, j * P:(j + 1) * P],
                                     rhs=sent_sb[:, k:k + 1],
                                     start=(k == 0), stop=(k == BERT // P - 1))
                bcol = sbA.tile([P, 1], dt.float32, tag="bcol")
                nc.sync.dma_start(bcol[:], fc0_b[j * P:(j + 1) * P][:, None])
                sc = sbA.tile([P, 1], dt.float32, tag="scol")
                nc.vector.tensor_add(sc[:], ps_v[:, 0:1], bcol[:])
                nc.scalar.activation(sc[:], sc[:], AF.Relu)
                nc.sync.dma_start(sent_dram[j * P:(j + 1) * P][:, None], sc[:])

            # W1aug = [conv1_W | W@a_src | W@a_dst] as 4 k-tiles [128, 1032]
            c1a_bc = sbA.tile([P, 2 * H * HC], dt.float32, tag="c1abc", bufs=1)
            nc.sync.dma_start(c1a_bc[:], bcast(c1a, 2 * H * HC))
            w1aug = [sbA.tile([P, XP1W], dt.float32, tag=f"w1aug{k}", bufs=1,
                              name=f"w1aug{k}")
                     for k in range(S // P)]
            for k in range(S // P):
                nc.sync.dma_start(w1aug[k][:, 0:H * HC],
                                  c1w[k * P:(k + 1) * P, :])
                tmp = sbA.tile([P, H * HC], dt.float32, tag="scratch4k")
                nc.vector.tensor_mul(tmp[:], w1aug[k][:, 0:H * HC],
                                     c1a_bc[:, 0:H * HC])
                for h in range(H):
                    nc.vector.reduce_sum(
                        w1aug[k][:, H * HC + h:H * HC + h + 1],
                        tmp[:, h * HC:(h + 1) * HC], axis=mybir.AxisListType.X)
                nc.vector.tensor_mul(tmp[:], w1aug[k][:, 0:H * HC],
                                     c1a_bc[:, H * HC:2 * H * HC])
                for h in range(H):
                    nc.vector.reduce_sum(
                        w1aug[k][:, H * HC + H + h:H * HC + H + h + 1],
                        tmp[:, h * HC:(h + 1) * HC], axis=mybir.AxisListType.X)

            _mark(nc, 'A:prep')
            # hT = (relu(x @ fc1_w + b1) + sent)^T computed directly per
            # s-chunk (lhsT = fc1_w block, rhs = xT block); xp1 = hT^T @ W1aug
            fc1w_t = [sbA.tile([P, S], dt.float32, tag=f"fc1w{k}", bufs=1,
                               name=f"fc1w{k}")
                      for k in range(F // P)]
            for k in range(F // P):
                nc.sync.dma_start(fc1w_t[k][:], fc1_w[k * P:(k + 1) * P, :])
            xT_sb = [single.tile([P, N], dt.float32, name=f"xT{k}")
                     for k in range(F // P)]
            for k in range(F // P):
                nc.sync.dma_start(xT_sb[k][:], xT_in[k * P:(k + 1) * P, :])
            w1aug_h = [sbA.tile([P, XP1W], dt.float16, tag=f"w1augh{k}", bufs=1,
                               name=f"w1augh{k}")
                       for k in range(S // P)]
            for k in range(S // P):
                nc.vector.tensor_copy(w1aug_h[k][:], w1aug[k][:])
            fc1b_col = single.tile([P, S // P], dt.float32)
            nc.sync.dma_start(fc1b_col[:],
                              fc1_b.ap().rearrange("(j p) -> p j", p=P))
            sent_col = single.tile([P, S // P], dt.float32)
            nc.sync.dma_start(sent_col[:],
                              sent_dram[:].rearrange("(j p) -> p j", p=P))
            for nt in range(N // P):
                hT_t = []
                for sj in range(S // P):
                    ps_hT = psum2.tile([P, P], dt.float32, space="PSUM",
                                       tag="ps_xt")
                    for k in range(F // P):
                        nc.tensor.matmul(
                            ps_hT[:], lhsT=fc1w_t[k][:, sj * P:(sj + 1) * P],
                            rhs=xT_sb[k][:, nt * P:(nt + 1) * P],
                            start=(k == 0), stop=(k == F // P - 1))
                    ht = sbA.tile([P, P], dt.float16, tag=f"hT{sj}",
                                  name=f"hT{sj}")
                    nc.scalar.activation(ht[:], ps_hT[:], AF.Relu,
                                         bias=fc1b_col[:, sj:sj + 1])
                    nc.vector.tensor_tensor(
                        ht[:], ht[:],
                        sent_col[:, sj:sj + 1].to_broadcast([P, P]),
                        op=ALU.add)
                    hT_t.append(ht)
                ps_xp1 = psA.tile([P, XP1W], dt.float32, space="PSUM", tag="ps_xp1")
                for sj in range(S // P):
                    for s0, s1 in ((0, 512), (512, 1024), (1024, XP1W)):
                        nc.tensor.matmul(ps_xp1[:, s0:s1], lhsT=hT_t[sj][:],
                                         rhs=w1aug_h[sj][:, s0:s1],
                                         start=(sj == 0), stop=(sj == S // P - 1))
                xp1_t = sbA.tile([P, XP1W], dt.float16, tag="xp1")
                nc.vector.tensor_copy(xp1_t[:], ps_xp1[:])
                nc.sync.dma_start(xp1_dram[nt * P:(nt + 1) * P, :], xp1_t[:])
                nc.sync.dma_start(al1d_dram[nt * P:(nt + 1) * P, :],
                                  xp1_t[:, H * HC:H * HC + 2 * H])
                if debug:
                    nc.sync.dma_start(dbg["xp1_dbg"][nt * P:(nt + 1) * P, :],
                                      xp1_t[:])

            _mark(nc, 'A:h_xp1')
            # g1/g2 via split-fp16 (hi+lo) wide-N matmuls: w.e ~= wh.eh +
            # wh.el + wl.eh (exact to ~1e-7 rel); row-contiguous weight loads
            elph_sb = single.tile([P, E // P], dt.float16)
            nc.sync.dma_start(elph_sb[:],
                              elph_in.ap().rearrange("(k p) -> p k", p=P))
            elpl_sb = single.tile([P, E // P], dt.float16)
            nc.sync.dma_start(elpl_sb[:],
                              elpl_in.ap().rearrange("(k p) -> p k", p=P))
            sent_h = single.tile([P, BERT // P], dt.float16)
            nc.vector.tensor_copy(sent_h[:], sent_sb[:])
            sent_hb = single.tile([P, BERT // P], dt.float32)
            nc.vector.tensor_copy(sent_hb[:], sent_h[:])
            sent_l = single.tile([P, BERT // P], dt.float16)
            nc.vector.tensor_sub(sent_l[:], sent_sb[:], sent_hb[:])
            g_sb = single.tile([P, 2 * ECH // P], dt.float32)
            EC2 = ECH // 2
            ps_grow = psA.tile([1, ECH], dt.float32, space="PSUM", tag="grow",
                               bufs=1, name="ps_grow")
            KB = 4                        # k-tiles per weight DMA
            for kq in range(E // P // KB):
                wqh = sbA.tile([P, KB, ECH], dt.float16, tag="wcolh", bufs=2,
                               name="wqh")
                nc.sync.dma_start(
                    wqh[:], fclwh_sh[kq * KB * P:(kq + 1) * KB * P, :]
                    .rearrange("(k p) j -> p k j", p=P))
                wql = sbA.tile([P, KB, ECH], dt.float16, tag="wcoll", bufs=2,
                               name="wql")
                nc.sync.dma_start(
                    wql[:], fclwl_sh[kq * KB * P:(kq + 1) * KB * P, :]
                    .rearrange("(k p) j -> p k j", p=P))
                for kk in range(KB):
                    k = kq * KB + kk
                    for hh in range(2):
                        reg = ps_grow[:, hh * EC2:(hh + 1) * EC2]
                        wh = wqh[:, kk, hh * EC2:(hh + 1) * EC2]
                        wl = wql[:, kk, hh * EC2:(hh + 1) * EC2]
                        nc.tensor.matmul(reg, lhsT=elph_sb[:, k:k + 1], rhs=wh,
                                         start=(k == 0), stop=False)
                        nc.tensor.matmul(reg, lhsT=elpl_sb[:, k:k + 1], rhs=wh,
                                         start=False, stop=False)
                        nc.tensor.matmul(reg, lhsT=elph_sb[:, k:k + 1], rhs=wl,
                                         start=False,
                                         stop=(k == E // P - 1))
            g1row = sbA.tile([1, ECH], dt.float32, tag="g1row", bufs=1)
            nc.vector.tensor_copy(g1row[:], ps_grow[:])
            nc.sync.dma_start(graw[0:ECH][None, :], g1row[:])
            ps_grow2 = psA.tile([1, ECH], dt.float32, space="PSUM", tag="grow",
                                bufs=1, name="ps_grow")
            for kq in range(BERT // P // 2):
                wqh = sbA.tile([P, KB, ECH], dt.float16, tag="wcolh", bufs=2,
                               name="wqh")
                nc.sync.dma_start(
                    wqh[:, 0:2, :], fcewh_sh[kq * 2 * P:(kq + 1) * 2 * P, :]
                    .rearrange("(k p) j -> p k j", p=P))
                wql = sbA.tile([P, KB, ECH], dt.float16, tag="wcoll", bufs=2,
                               name="wql")
                nc.sync.dma_start(
                    wql[:, 0:2, :], fcewl_sh[kq * 2 * P:(kq + 1) * 2 * P, :]
                    .rearrange("(k p) j -> p k j", p=P))
                for kk in range(2):
                    k = kq * 2 + kk
                    for hh in range(2):
                        reg = ps_grow2[:, hh * EC2:(hh + 1) * EC2]
                        wh = wqh[:, kk, hh * EC2:(hh + 1) * EC2]
                        wl = wql[:, kk, hh * EC2:(hh + 1) * EC2]
                        nc.tensor.matmul(reg, lhsT=sent_h[:, k:k + 1], rhs=wh,
                                         start=(k == 0), stop=False)
                        nc.tensor.matmul(reg, lhsT=sent_l[:, k:k + 1], rhs=wh,
                                         start=False, stop=False)
                        nc.tensor.matmul(reg, lhsT=sent_h[:, k:k + 1], rhs=wl,
                                         start=False,
                                         stop=(k == BERT // P - 1))
            g2row = sbA.tile([1, ECH], dt.float32, tag="g2row", bufs=1)
            nc.vector.tensor_copy(g2row[:], ps_grow2[:])
            nc.sync.dma_start(graw[ECH:2 * ECH][None, :], g2row[:])
            gcol_raw = single.tile([P, 2 * ECH // P], dt.float32)
            nc.sync.dma_start(gcol_raw[:],
                              graw[:].rearrange("(j p) -> p j", p=P))
            b1col = sbA.tile([P, ECH // P], dt.float32, tag="b1col", bufs=1)
            nc.sync.dma_start(b1col[:],
                              fclb_sh.ap().rearrange("(j p) -> p j", p=P))
            b2col = sbA.tile([P, ECH // P], dt.float32, tag="b2col", bufs=1)
            nc.sync.dma_start(b2col[:],
                              fceb_sh.ap().rearrange("(j p) -> p j", p=P))
            nc.vector.tensor_add(g_sb[:, 0:ECH // P], gcol_raw[:, 0:ECH // P],
                                 b1col[:])
            nc.vector.tensor_add(g_sb[:, ECH // P:], gcol_raw[:, ECH // P:],
                                 b2col[:])
            nc.scalar.activation(g_sb[:], g_sb[:], AF.Relu)
            _mark(nc, 'B:g12')
            # ---- zero-skip fc2: sparse_gather-compact nonzero g rows ----
            NGJ = 2 * ECH // P            # 16 g columns in g_sb
            NT2 = NNZ_PAD // P            # 10 gathered k-tiles
            iota1_sb = single.tile([P, NGJ], dt.float32)
            nc.sync.dma_start(iota1_sb[:],
                              iota1_in.ap().rearrange("(j p) -> p j", p=P))
            posl_sb = single.tile([P, NT2], dt.float32)
            nc.sync.dma_start(posl_sb[:],
                              posl_in.ap().rearrange("(t q) -> q t", q=P))
            # write g (+0 sentinel) to DRAM for value gathers
            nc.sync.dma_start(
                g12ext[0:2 * ECH, :].rearrange("(j p) x -> p (j x)", p=P),
                g_sb[:])
            zsent = sbA.tile([1, 1], dt.float32, tag="zsent")
            nc.vector.memset(zsent[:], 0.0)
            nc.sync.dma_start(g12ext[2 * ECH:2 * ECH + 1, 0][None, :],
                              zsent[:])
            # enc = k if g[k] > 0 else -1   (k = j*128+p)
            nz = sbA.tile([P, NGJ], dt.float32, tag="nz")
            nc.vector.tensor_scalar(nz[:], g_sb[:], 0.0, None, op0=ALU.is_gt)
            enc = sbA.tile([P, NGJ], dt.float32, tag="enc")
            nc.vector.tensor_mul(enc[:], nz[:], iota1_sb[:])
            nc.vector.tensor_scalar_add(enc[:], enc[:], -1.0)
            nc.sync.dma_start(encd[:].rearrange("(j p) x -> p (j x)", p=P),
                              enc[:])
            src16 = sbA.tile([16, P * NGJ // 16], dt.float32, tag="src16",
                             bufs=1)
            nc.sync.dma_start(src16[:], encd[:].rearrange("(f p) x -> p (f x)",
                                                          p=16))
            ks16 = sbA.tile([16, NNZ_PAD // 16], dt.float32, tag="ks16", bufs=1)
            nf16 = sbA.tile([1, 1], dt.uint32, tag="nf16")
            nc.gpsimd.sparse_gather(ks16[:], src16[:], num_found=nf16[:])
            nc.sync.dma_start(
                klist_dram[:].rearrange("(f p) x -> p (f x)", p=16), ks16[:])
            klf = sbA.tile([P, NT2], dt.float32, tag="klf", bufs=1)
            nc.sync.dma_start(
                klf[:], klist_dram[:].rearrange("(t q) x -> q (t x)", q=P))
            nff = sbA.tile([1, 1], dt.float32, tag="nff")
            nc.vector.tensor_copy(nff[:], nf16[:])
            nc.sync.dma_start(row_dram[None, 3 * HC:3 * HC + 1], nff[:])
            nfb = sbA.tile([P, 1], dt.float32, tag="nfb", bufs=1)
            nc.sync.dma_start(nfb[:], bcast_ap(row_dram, 1, 3 * HC))
            pred = sbA.tile([P, NT2], dt.uint8, tag="pred")
            nc.vector.tensor_tensor(pred[:], posl_sb[:],
                                    nfb[:].to_broadcast([P, NT2]), op=ALU.is_lt)
            ksent = sbA.tile([P, NT2], dt.float32, tag="ksent")
            nc.vector.memset(ksent[:], float(2 * ECH))
            ksel = sbA.tile([P, NT2], dt.float32, tag="ksel")
            nc.vector.select(ksel[:], pred[:], klf[:], ksent[:])
            klist_i = single.tile([P, NT2], dt.int32)
            nc.vector.tensor_copy(klist_i[:], ksel[:])
            # gather g values for the compacted rows
            gl_sb = single.tile([P, NT2], dt.float32)
            for t in range(NT2):
                nc.gpsimd.indirect_dma_start(
                    out=gl_sb[:, t:t + 1], out_offset=None, in_=g12ext[:],
                    in_offset=bass.IndirectOffsetOnAxis(
                        ap=klist_i[:, t:t + 1], axis=0))
            # gather nonzero rows of fc2w and accumulate partial logits
            EH = E // 2
            f2acc = sbA.tile([P, E // P], dt.float32, tag="f2acc", bufs=1,
                             name="f2acc")
            for t in range(NT2):
                ps_t = psA.tile([P, E // P], dt.float32, space="PSUM",
                                tag="vec", bufs=1, name="ps_v")
                for half in range(2):
                    wg = sbA.tile([P, EH], dt.float32, tag="wg", bufs=2,
                                  name="wg")
                    nc.gpsimd.indirect_dma_start(
                        out=wg[:], out_offset=None, in_=fc2w_sh[:],
                        in_offset=bass.IndirectOffsetOnAxis(
                            ap=klist_i[:, t:t + 1], axis=0),
                        element_offset=half * EH)
                    for jj in range(EH // P):
                        j = half * (EH // P) + jj
                        nc.tensor.matmul(ps_t[:, j:j + 1],
                                         lhsT=wg[:, jj * P:(jj + 1) * P],
                                         rhs=gl_sb[:, t:t + 1],
                                         start=True, stop=True)
                if t == 0:
                    nc.vector.tensor_copy(f2acc[:], ps_t[:])
                else:
                    nc.vector.tensor_add(f2acc[:], f2acc[:], ps_t[:])
            nc.sync.dma_start(fc2part[:].rearrange("(j p) -> p j", p=P),
                              f2acc[:])
            nc.sync.dma_start(kdbg_out.ap().rearrange("(t p) -> p t", p=P),
                              klist_i[:])
            nc.sync.dma_start(gdbg_out.ap().rearrange("(t p) -> p t", p=P),
                              gl_sb[:])
            nc.sync.dma_start(pdbg_out[:, :], enc[:])
            nc.sync.dma_start(g12dbg_out.ap().rearrange("(j p) -> p j", p=P),
                              g_sb[:])
            _mark(nc, 'B:fc2')
            esA.close()
            # ---- conv1 pass 1 (mask-independent): gathers, alpha, exp, msg ----
            gsrc_sb = single.tile([P, nt_g], dt.int32)
            nc.sync.dma_start(gsrc_sb[:],
                              g_src.ap().rearrange("(t p) -> p t", p=P))
            gdst_sb = single.tile([P, nt_g], dt.int32)
            nc.sync.dma_start(gdst_sb[:],
                              g_dst.ap().rearrange("(t p) -> p t", p=P))
            glidx_sb = single.tile([P, nt_g], dt.int32)
            nc.sync.dma_start(glidx_sb[:],
                              g_lidx.ap().rearrange("(t p) -> p t", p=P))
            esC1 = ExitStack()
            sbC1 = esC1.enter_context(tc.tile_pool(name="sbC1", bufs=2))
            for t in range(nt_g):
                xs = sbC1.tile([P, XP1W], dt.float16, tag="gxs", bufs=4)
                nc.gpsimd.indirect_dma_start(
                    out=xs[:], out_offset=None, in_=xp1_dram[:],
                    in_offset=bass.IndirectOffsetOnAxis(
                        ap=gsrc_sb[:, t:t + 1], axis=0))
                ad = sbC1.tile([P, 2 * H], dt.float16, tag="gad", bufs=4)
                nc.gpsimd.indirect_dma_start(
                    out=ad[:], out_offset=None, in_=al1d_dram[:],
                    in_offset=bass.IndirectOffsetOnAxis(
                        ap=gdst_sb[:, t:t + 1], axis=0))
                alpha = sbC1.tile([P, H], dt.float32, tag="alpha")
                nc.vector.tensor_add(alpha[:], xs[:, H * HC:H * HC + H],
                                     ad[:, H:2 * H])
                nc.vector.scalar_tensor_tensor(alpha[:], alpha[:], 0.2, alpha[:],
                                               op0=ALU.mult, op1=ALU.max)
                ex = sbC1.tile([P, H], dt.float32, tag="ex")
                nc.scalar.activation(ex[:], alpha[:], AF.Exp)
                if debug:
                    nc.sync.dma_start(dbg["xs_dbg"][t * P:(t + 1) * P, :], xs[:])
                    nc.sync.dma_start(dbg["ad_dbg"][t * P:(t + 1) * P, :], ad[:])
                msg = sbC1.tile([P, H * HC + H], dt.float16, tag="msg", bufs=4)
                for h in range(H):
                    nc.vector.tensor_tensor(
                        msg[:, h * HC:(h + 1) * HC], xs[:, h * HC:(h + 1) * HC],
                        ex[:, h:h + 1].to_broadcast([P, HC]), op=ALU.mult)
                nc.vector.tensor_copy(msg[:, H * HC:H * HC + H], ex[:])
                nc.sync.dma_start(msg_dram[t * P:(t + 1) * P, :], msg[:])
            esC1.close()
            if stage >= 2:

                nc.gpsimd.collective_compute(
                    "AllReduce", ALU.add, replica_groups=RG,
                    ins=[fc2part[:]], outs=[logits_dram[:]])
                # logits += fc2_b ; orig_out ; logits_ext
                lg_pf = single.tile([P, E // P], dt.float32)
                nc.sync.dma_start(lg_pf[:], logits_dram[:].rearrange("(p f) -> p f", p=P))
                f2b_pf = single.tile([P, E // P], dt.float32)
                nc.sync.dma_start(f2b_pf[:], fc2_b.ap().rearrange("(p f) -> p f", p=P))
                nc.vector.tensor_add(lg_pf[:], lg_pf[:], f2b_pf[:])
                nc.sync.dma_start(orig_out.ap().rearrange("(p f) -> p f", p=P), lg_pf[:])
                nc.sync.dma_start(
                    lext_dram[0:E, :].rearrange("(p f) x -> p (f x)", p=P), lg_pf[:])
                big_t = single.tile([1, 2], dt.float32)
                nc.vector.memset(big_t[:, 0:1], BIG)
                nc.vector.memset(big_t[:, 1:2], -BIG)
                nc.sync.dma_start(lext_dram[E:E + 2, 0][None, :], big_t[:])

                _mark(nc, 'AR:logits')
                # ==== conv1 pass 2: valid folded into one-hot, aggregate ====
                valid_t = [keep.tile([P, 1], dt.float32, tag=f"valid{t}",
                                     name=f"valid{t}")
                           for t in range(nt_g)]
                esC = ExitStack()
                sbC = esC.enter_context(tc.tile_pool(name="sbC", bufs=2))
                psC = esC.enter_context(tc.tile_pool(name="psC", bufs=1, space="PSUM"))
                ps_msg = [psC.tile([P, H * HC], dt.float32, space="PSUM",
                                   tag=f"ps_msg{d}", name=f"ps_msg{d}")
                          for d in range(2)]
                ps_den = [psC.tile([P, H], dt.float32, space="PSUM",
                                   tag=f"ps_den{d}", name=f"ps_den{d}")
                          for d in range(2)]
                for t in range(nt_g):
                    oh_t = sbC.tile([P, NCHUNK], dt.float16, tag="oh", bufs=4)
                    nc.sync.dma_start(oh_t[:], g_oh[t * P:(t + 1) * P, :])
                    lg = sbC.tile([P, 1], dt.float32, tag="glg", bufs=4)
                    nc.gpsimd.indirect_dma_start(
                        out=lg[:], out_offset=None, in_=lext_dram[:],
                        in_offset=bass.IndirectOffsetOnAxis(
                            ap=glidx_sb[:, t:t + 1], axis=0))
                    nc.vector.tensor_scalar(valid_t[t][:], lg[:], 0.0, None,
                                            op0=ALU.is_gt)
                    ohv = sbC.tile([P, NCHUNK], dt.float16, tag="ohv", bufs=4)
                    nc.vector.tensor_tensor(
                        ohv[:], oh_t[:], valid_t[t][:].to_broadcast([P, NCHUNK]),
                        op=ALU.mult)
                    msg = sbC.tile([P, H * HC + H], dt.float16, tag="msg",
                                   bufs=4)
                    nc.sync.dma_start(msg[:], msg_dram[t * P:(t + 1) * P, :])
                    for d in range(2):
                        lhsT = ohv[:, d * P:(d + 1) * P]
                        st, sp = (t == 0), (t == nt_g - 1)
                        nc.tensor.matmul(ps_msg[d][:, 0:512], lhsT=lhsT,
                                         rhs=msg[:, 0:512], start=st, stop=sp)
                        nc.tensor.matmul(ps_msg[d][:, 512:1024], lhsT=lhsT,
                                         rhs=msg[:, 512:1024], start=st, stop=sp)
                        nc.tensor.matmul(ps_den[d][:], lhsT=lhsT,
                                         rhs=msg[:, H * HC + H - H:H * HC + H],
                                         start=st, stop=sp)
                _mark(nc, 'C:conv1agg')
                # finalize conv1 + xp2aug
                c1b_bc = sbC.tile([P, H * HC], dt.float32, tag="c1bbc", bufs=1)
                nc.sync.dma_start(c1b_bc[:], bcast(c1b, H * HC))
                c2a_bc = sbC.tile([P, 2 * F], dt.float32, tag="c2abc", bufs=1)
                nc.sync.dma_start(c2a_bc[:], bcast(c2a, 2 * F))
                w2aug = [keep.tile([P, XP2W], dt.float32, tag=f"w2aug{k}",
                                   name=f"w2aug{k}")
                         for k in range(H * HC // P)]
                for k in range(H * HC // P):
                    nc.sync.dma_start(w2aug[k][:, 0:F], c2w[k * P:(k + 1) * P, :])
                    tmp = sbC.tile([P, F], dt.float32, tag="w2tmp")
                    nc.vector.tensor_mul(tmp[:], w2aug[k][:, 0:F], c2a_bc[:, 0:F])
                    nc.vector.reduce_sum(w2aug[k][:, F:F + 1], tmp[:],
                                         axis=mybir.AxisListType.X)
                    nc.vector.tensor_mul(tmp[:], w2aug[k][:, 0:F], c2a_bc[:, F:2 * F])
                    nc.vector.reduce_sum(w2aug[k][:, F + 1:F + 2], tmp[:],
                                         axis=mybir.AxisListType.X)
                if debug:
                    for d in range(2):
                        dd = sbC.tile([P, H], dt.float32, tag="dendbg")
                        nc.vector.tensor_copy(dd[:], ps_den[d][:])
                        nc.sync.dma_start(dbg["den_dbg"][d * P:(d + 1) * P, :], dd[:])
                        dm = sbC.tile([P, H * HC], dt.float32, tag="msumdbg")
                        nc.vector.tensor_copy(dm[:], ps_msg[d][:])
                        nc.sync.dma_start(dbg["msum_dbg"][d * P:(d + 1) * P, :], dm[:])
                h1_keep = [keep.tile([P, H * HC], dt.float16, tag=f"h1k{d}",
                                     name=f"h1k{d}")
                           for d in range(2)]
                for d in range(2):
                    denr = sbC.tile([P, H], dt.float32, tag="denr")
                    nc.vector.reciprocal(denr[:], ps_den[d][:])
                    h1_t = h1_keep[d]
                    for h in range(H):
                        nc.vector.scalar_tensor_tensor(
                            h1_t[:, h * HC:(h + 1) * HC],
                            ps_msg[d][:, h * HC:(h + 1) * HC],
                            denr[:, h:h + 1],
                            c1b_bc[:, h * HC:(h + 1) * HC],
                            op0=ALU.mult, op1=ALU.add)
                    # elu = relu(x) + exp(min(x,0)) - 1
                    relu_t = sbC.tile([P, H * HC], dt.float32, tag="elu_r")
                    nc.scalar.activation(relu_t[:], h1_t[:], AF.Relu)
                    nc.vector.tensor_scalar_min(h1_t[:], h1_t[:], 0.0)
                    nc.scalar.activation(h1_t[:], h1_t[:], AF.Exp)
                    nc.vector.scalar_tensor_tensor(h1_t[:], h1_t[:], -1.0,
                                                   relu_t[:],
                                                   op0=ALU.add, op1=ALU.add)
                    if debug:
                        nc.sync.dma_start(dbg["h1_dbg"][d * P:(d + 1) * P, :], h1_t[:])
                esC.close()
                esD = ExitStack()
                sbD = esD.enter_context(tc.tile_pool(name="sbD", bufs=2))
                psD = esD.enter_context(tc.tile_pool(name="psD", bufs=1, space="PSUM"))
                w2aug_h = [sbD.tile([P, XP2W], dt.float16, tag=f"w2augh{k}", bufs=1,
                                    name=f"w2augh{k}")
                           for k in range(H * HC // P)]
                for k in range(H * HC // P):
                    nc.vector.tensor_copy(w2aug_h[k][:], w2aug[k][:])
                for d in range(2):
                    h1_t = h1_keep[d]
                    ps_xp2 = psD.tile([P, XP2W], dt.float32, space="PSUM", tag="ps_xp2")
                    for k in range(H * HC // P):
                        ps_h1t = psD.tile([P, P], dt.float16, space="PSUM",
                                          tag="ps_xth", bufs=2)
                        nc.tensor.transpose(ps_h1t[:], h1_t[:, k * P:(k + 1) * P],
                                            ident_h[:])
                        h1T = sb.tile([P, P], dt.float16, tag="xTh")
                        nc.vector.tensor_copy(h1T[:], ps_h1t[:])
                        nc.tensor.matmul(ps_xp2[:], lhsT=h1T[:], rhs=w2aug_h[k][:],
                                         start=(k == 0), stop=(k == H * HC // P - 1))
                    xp2_t = sbD.tile([P, XP2W], dt.float16, tag="xp2")
                    nc.vector.tensor_copy(xp2_t[:], ps_xp2[:])
                    nc.sync.dma_start(xp2_in[d * P:(d + 1) * P, :], xp2_t[:])
                _mark(nc, 'D:xp2')
                esD.close()
                nc.gpsimd.collective_compute(
                    "AllGather", ALU.bypass, replica_groups=RG,
                    ins=[xp2_in[:]], outs=[xp2_dram[:]])
                if debug:
                    for nt in range(N // P):
                        dtmp = sb.tile([P, XP2W], dt.float16, tag="dbg1")
                        nc.sync.dma_start(dtmp[:], xp2_dram[nt * P:(nt + 1) * P, :])
                        nc.sync.dma_start(dbg["xp2_dbg"][nt * P:(nt + 1) * P, :],
                                          dtmp[:])

            _mark(nc, 'AG:xp2')
            if stage >= 3:
                # ============ conv2 aggregation ============
                esE = ExitStack()
                sbE = esE.enter_context(tc.tile_pool(name="sbE", bufs=2))
                psE = esE.enter_context(tc.tile_pool(name="psE", bufs=1, space="PSUM"))
                ps_m2 = [psE.tile([P, F + 1], dt.float32, space="PSUM",
                                  tag=f"ps_m2{d}", name=f"ps_m2{d}")
                         for d in range(2)]
                for t in range(nt_g):
                    oh_t = sbE.tile([P, NCHUNK], dt.float16, tag="oh2", bufs=4)
                    nc.sync.dma_start(oh_t[:], g_oh[t * P:(t + 1) * P, :])
                    xs2 = sbE.tile([P, XP2W], dt.float16, tag="xs2", bufs=4)
                    nc.gpsimd.indirect_dma_start(
                        out=xs2[:], out_offset=None, in_=xp2_dram[:],
                        in_offset=bass.IndirectOffsetOnAxis(
                            ap=gsrc_sb[:, t:t + 1], axis=0))
                    xd2 = sbE.tile([P, XP2W], dt.float16, tag="xd2", bufs=4)
                    nc.gpsimd.indirect_dma_start(
                        out=xd2[:], out_offset=None, in_=xp2_dram[:],
                        in_offset=bass.IndirectOffsetOnAxis(
                            ap=gdst_sb[:, t:t + 1], axis=0))
                    alpha2 = sbE.tile([P, 1], dt.float32, tag="alpha2")
                    nc.vector.tensor_add(alpha2[:], xs2[:, F:F + 1],
                                         xd2[:, F + 1:F + 2])
                    nc.vector.scalar_tensor_tensor(alpha2[:], alpha2[:], 0.2, alpha2[:],
                                                   op0=ALU.mult, op1=ALU.max)
                    ex2 = sbE.tile([P, 1], dt.float32, tag="ex2")
                    nc.scalar.activation(ex2[:], alpha2[:], AF.Exp)
                    nc.vector.tensor_mul(ex2[:], ex2[:], valid_t[t][:])
                    msg2 = sbE.tile([P, F + 1], dt.float16, tag="msg2")
                    nc.vector.tensor_tensor(msg2[:, 0:F], xs2[:, 0:F],
                                            ex2[:].to_broadcast([P, F]), op=ALU.mult)
                    nc.vector.tensor_copy(msg2[:, F:F + 1], ex2[:])
                    for d in range(2):
                        lhsT = oh_t[:, d * P:(d + 1) * P]
                        st, sp = (t == 0), (t == nt_g - 1)
                        nc.tensor.matmul(ps_m2[d][:], lhsT=lhsT, rhs=msg2[:],
                                         start=st, stop=sp)
                _mark(nc, 'E:conv2agg')
                c2b_bc = sbE.tile([P, F], dt.float32, tag="c2bbc", bufs=1)
                nc.sync.dma_start(c2b_bc[:], bcast(c2b, F))
                for d in range(2):
                    d2r = sbE.tile([P, 1], dt.float32, tag="d2r")
                    nc.vector.reciprocal(d2r[:], ps_m2[d][:, F:F + 1])
                    h2_t = sbE.tile([P, F], dt.float16, tag="h2")
                    nc.vector.tensor_tensor(h2_t[:], ps_m2[d][:, 0:F],
                                            d2r[:].to_broadcast([P, F]), op=ALU.mult)
                    nc.vector.tensor_add(h2_t[:], h2_t[:], c2b_bc[:])
                    nc.sync.dma_start(h2_in[d * P:(d + 1) * P, :], h2_t[:])
                esE.close()
                nc.gpsimd.collective_compute(
                    "AllGather", ALU.bypass, replica_groups=RG,
                    ins=[h2_in[:]], outs=[h2_dram[:]])
                if debug:
                    for nt in range(N // P):
                        dtmp2 = sb.tile([P, F], dt.float16, tag="dbg2")
                        nc.sync.dma_start(dtmp2[:], h2_dram[nt * P:(nt + 1) * P, :])
                        nc.sync.dma_start(dbg["h2_dbg"][nt * P:(nt + 1) * P, :],
                                          dtmp2[:])

            _mark(nc, 'AG:h2+fin')
            if stage >= 4:
                # ============ edge MLP ============
                msrc_sb = single.tile([P, nt_m], dt.int32)
                nc.sync.dma_start(msrc_sb[:], m_src.ap().rearrange("(t p) -> p t", p=P))
                mdst_sb = single.tile([P, nt_m], dt.int32)
                nc.sync.dma_start(mdst_sb[:], m_dst.ap().rearrange("(t p) -> p t", p=P))
                mlidx_sb = single.tile([P, nt_m], dt.int32)
                nc.sync.dma_start(mlidx_sb[:], m_lidx.ap().rearrange("(t p) -> p t", p=P))
                m1w_t = [keep.tile([P, HC], dt.float32, tag=f"m1w{k}", name=f"m1w{k}")
                         for k in range(4 * F // P)]
                m1w_h = [keep.tile([P, HC], dt.float16, tag=f"m1wh{k}",
                                   name=f"m1wh{k}")
                         for k in range(4 * F // P)]
                for k in range(4 * F // P):
                    nc.sync.dma_start(m1w_t[k][:], m1w[k * P:(k + 1) * P, :])
                    nc.vector.tensor_copy(m1w_h[k][:], m1w_t[k][:])
                m1b_bc = single.tile([P, HC], dt.float32)
                nc.sync.dma_start(m1b_bc[:], bcast(m1b, HC))
                mask_f = [keep.tile([P, 1], dt.float32, tag=f"maskf{t}",
                                    name=f"maskf{t}")
                          for t in range(nt_m)]
                mask_u8 = [keep.tile([P, 1], dt.uint8, tag=f"masku{t}",
                                     name=f"masku{t}")
                           for t in range(nt_m)]
                z1_t = [keep.tile([P, HC], dt.float32, tag=f"z1_{t}", name=f"z1_{t}")
                        for t in range(nt_m)]
                esF = ExitStack()
                sbF = esF.enter_context(tc.tile_pool(name="sbF", bufs=2))
                psF = esF.enter_context(tc.tile_pool(name="psF", bufs=1, space="PSUM"))
                ps_s1 = psF.tile([1, HC], dt.float32, space="PSUM", tag="ps_s1")
                ps_q1 = psF.tile([1, HC], dt.float32, space="PSUM", tag="ps_q1")
                ps_cnt = psF.tile([1, 1], dt.float32, space="PSUM", tag="ps_cnt")
                for t in range(nt_m):
                    xi = sbF.tile([P, F], dt.float16, tag="xi", bufs=4)
                    nc.gpsimd.indirect_dma_start(
                        out=xi[:], out_offset=None, in_=h2_dram[:],
                        in_offset=bass.IndirectOffsetOnAxis(
                            ap=msrc_sb[:, t:t + 1], axis=0))
                    xj = sbF.tile([P, F], dt.float16, tag="xj", bufs=4)
                    nc.gpsimd.indirect_dma_start(
                        out=xj[:], out_offset=None, in_=h2_dram[:],
                        in_offset=bass.IndirectOffsetOnAxis(
                            ap=mdst_sb[:, t:t + 1], axis=0))
                    lg = sbF.tile([P, 1], dt.float32, tag="mlg", bufs=6)
                    nc.gpsimd.indirect_dma_start(
                        out=lg[:], out_offset=None, in_=lext_dram[:],
                        in_offset=bass.IndirectOffsetOnAxis(
                            ap=mlidx_sb[:, t:t + 1], axis=0))
                    nc.vector.tensor_scalar(mask_f[t][:], lg[:], 0.0, None,
                                            op0=ALU.is_gt)
                    nc.vector.tensor_copy(mask_u8[t][:], mask_f[t][:])
                    dsub = sbF.tile([P, F], dt.float16, tag="dsub")
                    nc.vector.tensor_sub(dsub[:], xi[:], xj[:])
                    nc.scalar.activation(dsub[:], dsub[:], AF.Abs)
                    pmul = sbF.tile([P, F], dt.float16, tag="pmul")
                    nc.vector.tensor_mul(pmul[:], xi[:], xj[:])
                    ps_z1 = psF.tile([P, HC], dt.float32, space="PSUM", tag="ps_z1")
                    for pi, piece in enumerate((xi, xj, dsub, pmul)):
                        for hf in range(2):
                            ps_t = psF.tile([P, P], dt.float16, space="PSUM",
                                            tag="ps_xth", bufs=2)
                            nc.tensor.transpose(ps_t[:], piece[:, hf * P:(hf + 1) * P],
                                                ident_h[:])
                            efT = sb.tile([P, P], dt.float16, tag="xTh")
                            nc.vector.tensor_copy(efT[:], ps_t[:])
                            k = pi * 2 + hf
                            nc.tensor.matmul(ps_z1[:], lhsT=efT[:],
                                             rhs=m1w_h[k][:],
                                             start=(k == 0), stop=(k == 7))
                    nc.vector.tensor_add(z1_t[t][:], ps_z1[:], m1b_bc[:])
                    if debug:
                        nc.sync.dma_start(dbg["z1_dbg"][t * P:(t + 1) * P, :],
                                          z1_t[t][:])
                    zsq = sbF.tile([P, HC], dt.float32, tag="zsq")
                    nc.vector.tensor_mul(zsq[:], z1_t[t][:], z1_t[t][:])
                    st, sp = (t == 0), (t == nt_m - 1)
                    nc.tensor.matmul(ps_s1[:], lhsT=mask_f[t][:], rhs=z1_t[t][:],
                                     start=st, stop=sp)
                    nc.tensor.matmul(ps_q1[:], lhsT=mask_f[t][:], rhs=zsq[:],
                                     start=st, stop=sp)
                    nc.tensor.matmul(ps_cnt[:], lhsT=mask_f[t][:], rhs=mask_f[t][:],
                                     start=st, stop=sp)
                _mark(nc, 'F:mlp1')
                # pack stats1, AllReduce
                s_sb = sbF.tile([1, HC], dt.float32, tag="stat")
                nc.vector.tensor_copy(s_sb[:], ps_s1[:])
                nc.sync.dma_start(st1_in[None, 0:HC], s_sb[:])
                q_sb = sbF.tile([1, HC], dt.float32, tag="stat")
                nc.vector.tensor_copy(q_sb[:], ps_q1[:])
                nc.sync.dma_start(st1_in[None, HC:2 * HC], q_sb[:])
                c_sb = sbF.tile([1, 1], dt.float32, tag="statc")
                nc.vector.tensor_copy(c_sb[:], ps_cnt[:])
                nc.sync.dma_start(st1_in[None, 2 * HC:2 * HC + 1], c_sb[:])
                zpad = sbF.tile([1, 7], dt.float32, tag="statz")
                nc.vector.memset(zpad[:], 0.0)
                nc.sync.dma_start(st1_in[None, 2 * HC + 1:520], zpad[:])
                esF.close()
